# revision 5
# baseline (speedup 1.0000x reference)
"""GAT layer (nn_GATLayer) on 8 Trainium2 NeuronCores via Bass/Tile.

Reference computation (N=8192, F=512, D=64):
    z = features @ W                      # [N, D]
    s = z @ a_self; t = z @ a_neigh       # [N, 1]
    e[i,j] = leakyrelu(s[i] + t[j], 0.2)
    attention = softmax(e + mask(A), axis=1)   # mask: -1e12 where A<=0
    h = attention @ z                     # [N, D]

Sharding: row-shard the N x N attention across 8 cores (1024 rows each).
Each core computes z for its own feature rows, all-gathers z augmented
with a ones column (so the softmax denominator falls out of the PV
matmul) and a t column, then streams its [8192 x 1024] transposed block
of A while computing E[j, i] and accumulating
    H_aug[d, i] = sum_j z_aug[j, d] * E[j, i]
on the PE. Row 64 of H_aug is the softmax denominator; the epilogue
transposes H_aug back, multiplies by its reciprocal, and stores h.

Key tricks:
  * Scores are kept transposed ([j partitions, i free]) so both the
    softmax reduction and the PV contraction run over j on the PE
    partition axis - no on-chip transposes of the big matrix.
  * exp(leakyrelu(s_i + t_j)) = max(exp(s_i)exp(t_j),
    exp(.2 s_i)exp(.2 t_j)) by monotonicity, and exp separates, so the
    per-tile work is one scaled copy (ACT), one fused mult+max (DVE
    scalar_tensor_tensor), and one mask multiply (DVE/GpSimd split) -
    no per-tile transcendentals beyond the prologue exponentials.
  * Masking multiplies by A in {0,1} after exp instead of adding -1e12
    before it (exactly zeroes masked entries); A arrives as float32.
  * PV matmuls run in float32r (full-rate fp32 on the PE).
  * z_aug rows are padded to 80 floats so collective/DMA rows stay
    64-byte aligned.
"""

import sys

sys.path.insert(0, "/opt/trn_rl_repo")

import numpy as np

N, F, D = 8192, 512, 64
NCORES = 8
R = N // NCORES          # rows per core (1024)
JC = N // 128            # j-chunks (64)
DP = D + 1               # z | ones  (65)
TCOL = D + 1             # t column index in padded z_aug (65)
ZW = 80                  # padded z_aug width (80 floats = 320B rows)
ALPHA = 0.2

# mask-multiply engine split: chunks with (jc % 8) < GPS_SPLIT go to GpSimd
GPS_SPLIT = 5

_CACHE = {}


def _build_program():
    import concourse.bacc as bacc
    import concourse.tile as tile
    from concourse import mybir
    from concourse.masks import make_identity

    f32 = mybir.dt.float32
    f32r = mybir.dt.float32r
    Alu = mybir.AluOpType
    Act = mybir.ActivationFunctionType

    nc = bacc.Bacc("TRN2", target_bir_lowering=False, debug=False, num_devices=NCORES)

    feat_t = nc.dram_tensor("feat_t", [F, R], f32, kind="ExternalInput")
    a_t = nc.dram_tensor("a_t", [N, R], f32, kind="ExternalInput")
    w_in = nc.dram_tensor("w", [F, D], f32, kind="ExternalInput")
    a_self = nc.dram_tensor("a_self", [D, 1], f32, kind="ExternalInput")
    a_neigh = nc.dram_tensor("a_neigh", [1, D], f32, kind="ExternalInput")
    h_out = nc.dram_tensor("h", [R, D], f32, kind="ExternalOutput")

    with tile.TileContext(nc) as tc:
        with (
            tc.tile_pool(name="const", bufs=1) as cst,
            tc.tile_pool(name="dram", bufs=1, space="DRAM") as dram,
            tc.tile_pool(name="ps_main", bufs=1, space="PSUM") as ps_main,
        ):
            # ---- prologue inputs ----
            ft = cst.tile([128, 4 * R], f32)        # features^T, F on partitions
            for c in range(4):
                nc.sync.dma_start(out=ft[:, c * R:(c + 1) * R],
                                  in_=feat_t[c * 128:(c + 1) * 128, :])
            w_sb = cst.tile([128, 4 * D], f32)
            for c in range(4):
                nc.sync.dma_start(out=w_sb[:, c * D:(c + 1) * D],
                                  in_=w_in[c * 128:(c + 1) * 128, :])
            asf = cst.tile([D, 1], f32)
            nc.sync.dma_start(out=asf[:], in_=a_self[:])
            anr = cst.tile([1, D], f32)
            nc.sync.dma_start(out=anr[:], in_=a_neigh[:])
            ones1 = cst.tile([1, 128], f32)
            nc.vector.memset(ones1[:], 1.0)

            hp = ps_main.tile([DP, R], f32)          # H_aug accumulator

            with tc.tile_pool(name="ps_pro", bufs=2, space="PSUM") as ps_pro:
                # a_neigh broadcast across partitions: [128, D]
                pan = ps_pro.tile([128, D], f32, tag="pro")
                nc.tensor.matmul(pan[:], ones1[:], anr[:], start=True, stop=True)
                anb = cst.tile([128, D], f32)
                nc.vector.tensor_copy(anb[:], pan[:])

                # ---- z_aug_local = [z | 1 | t | 0pad]  ([R, ZW]) ----
                zaug_local = dram.tile([R, ZW], f32)
                for ib in range(R // 128):
                    psz = ps_pro.tile([128, D], f32, tag="pro")
                    for c in range(4):
                        nc.tensor.matmul(
                            psz[:],
                            ft[:, c * R + ib * 128: c * R + (ib + 1) * 128],
                            w_sb[:, c * D:(c + 1) * D],
                            start=(c == 0), stop=(c == 3),
                        )
                    zb = cst.tile([128, ZW], f32, tag="zb")
                    nc.vector.memset(zb[:], 0.0)
                    nc.vector.tensor_copy(zb[:, 0:D], psz[:])
                    nc.vector.memset(zb[:, D:D + 1], 1.0)
                    # t for this block into column TCOL
                    tscr = cst.tile([128, D], f32, tag="tscr")
                    nc.vector.tensor_tensor(tscr[:], zb[:, 0:D], anb[:], Alu.mult)
                    nc.vector.tensor_reduce(
                        zb[:, TCOL:TCOL + 1], tscr[:], mybir.AxisListType.X, Alu.add)
                    nc.sync.dma_start(
                        out=zaug_local[ib * 128:(ib + 1) * 128, :], in_=zb[:])

                # ---- all-gather z_aug (one 64B-aligned collective) ----
                zaug_full = dram.tile([N, ZW], f32, addr_space="Shared")
                nc.gpsimd.collective_compute(
                    "AllGather", Alu.bypass,
                    replica_groups=[list(range(NCORES))],
                    ins=[zaug_local.opt()], outs=[zaug_full.opt()],
                )
                zf = cst.tile([128, JC, ZW], f32)    # z_aug, j-chunked
                nc.sync.dma_start(
                    out=zf[:],
                    in_=zaug_full[:].rearrange("(c p) d -> p c d", p=128))
                zf_r = cst.tile([128, JC, DP], f32r)  # f32r copy for PE stationary
                nc.vector.tensor_copy(zf_r[:], zf[:, :, 0:DP])

                # exp(t) and exp(alpha*t) per chunk: [128, JC]
                et1 = cst.tile([128, JC], f32)
                nc.scalar.activation(et1[:], zf[:, :, TCOL], Act.Exp)
                et2 = cst.tile([128, JC], f32)
                nc.scalar.activation(et2[:], zf[:, :, TCOL], Act.Exp, scale=ALPHA)

                # ---- z_local^T (D on partitions) -> s row -> broadcasts ----
                pzt = ps_pro.tile([D, R], f32, tag="pro")
                for c in range(4):
                    for hh in range(2):
                        nc.tensor.matmul(
                            pzt[:, hh * 512:(hh + 1) * 512],
                            w_sb[:, c * D:(c + 1) * D],
                            ft[:, c * R + hh * 512: c * R + (hh + 1) * 512],
                            start=(c == 0), stop=(c == 3),
                        )
                zt_sb = cst.tile([D, R], f32)
                nc.vector.tensor_copy(zt_sb[:], pzt[:])

                pss = ps_pro.tile([1, R], f32, tag="pro")
                for hh in range(2):
                    nc.tensor.matmul(
                        pss[:, hh * 512:(hh + 1) * 512],
                        asf[:],
                        zt_sb[:, hh * 512:(hh + 1) * 512],
                        start=True, stop=True,
                    )
                s_sb = cst.tile([1, R], f32)
                nc.vector.tensor_copy(s_sb[:], pss[:])
                psb = ps_pro.tile([128, R], f32, tag="pro")
                for hh in range(2):
                    nc.tensor.matmul(
                        psb[:, hh * 512:(hh + 1) * 512],
                        ones1[:],
                        s_sb[0:1, hh * 512:(hh + 1) * 512],
                        start=True, stop=True,
                    )
                s_bcast = cst.tile([128, R], f32)
                nc.vector.tensor_copy(s_bcast[:], psb[:])

                # exp(s) and exp(alpha*s), broadcast across partitions
                p1 = cst.tile([128, R], f32)
                nc.scalar.activation(p1[:], s_bcast[:], Act.Exp)
                p2 = cst.tile([128, R], f32)
                nc.scalar.activation(p2[:], s_bcast[:], Act.Exp, scale=ALPHA)

            # ---- main loop over j-chunks ----
            # E[j,i] = max(exp(s_i)exp(t_j), exp(.2 s_i)exp(.2 t_j)) * A[i,j]
            with (
                tc.tile_pool(name="a_pool", bufs=6) as a_pool,
                tc.tile_pool(name="work", bufs=3) as work,
            ):
                for jc in range(JC):
                    at = a_pool.tile([128, R], f32, tag="at")
                    nc.sync.dma_start(
                        out=at[:], in_=a_t[jc * 128:(jc + 1) * 128, :])

                    y2 = work.tile([128, R], f32, tag="y2")
                    nc.scalar.activation(
                        y2[:], p2[:], Act.Identity, scale=et2[:, jc:jc + 1])
                    e0 = work.tile([128, R], f32, tag="e0")
                    nc.vector.scalar_tensor_tensor(
                        e0[:], p1[:], et1[:, jc:jc + 1], y2[:],
                        Alu.mult, Alu.max)
                    ea = work.tile([128, R], f32r, tag="ea")
                    if jc % 8 < GPS_SPLIT:
                        nc.gpsimd.tensor_tensor(ea[:], e0[:], at[:], Alu.mult)
                    else:
                        nc.vector.tensor_tensor(ea[:], e0[:], at[:], Alu.mult)

                    for hh in range(2):
                        nc.tensor.matmul(
                            hp[:, hh * 512:(hh + 1) * 512],
                            zf_r[:, jc],
                            ea[:, hh * 512:(hh + 1) * 512],
                            start=(jc == 0), stop=(jc == JC - 1),
                        )

            # ---- epilogue: transpose H_aug, normalize, store ----
            with (
                tc.tile_pool(name="ps_epi", bufs=2, space="PSUM") as ps_epi,
                tc.tile_pool(name="epi", bufs=2) as epi,
            ):
                h_sb = cst.tile([DP, R], f32)
                nc.vector.tensor_copy(h_sb[:], hp[:])
                ident = cst.tile([DP, DP], f32)
                make_identity(nc, ident[:])
                for b in range(R // 128):
                    trp = ps_epi.tile([128, DP], f32, tag="trp")
                    nc.tensor.transpose(
                        trp[:], h_sb[:, b * 128:(b + 1) * 128], ident[:])
                    rec = epi.tile([128, 1], f32, tag="rec")
                    nc.vector.reciprocal(rec[:], trp[:, D:DP])
                    hb = epi.tile([128, D], f32, tag="hb")
                    nc.vector.tensor_scalar_mul(hb[:], trp[:, 0:D], rec[:, 0:1])
                    nc.sync.dma_start(
                        out=h_out[b * 128:(b + 1) * 128, :], in_=hb[:])

    nc.compile()
    return nc


def _get_program():
    if "nc" not in _CACHE:
        _CACHE["nc"] = _build_program()
    return _CACHE["nc"]


def kernel(features, A, W, a_self, a_neigh):
    from concourse.bass_utils import run_bass_kernel_spmd

    nc = _get_program()

    features = np.asarray(features, dtype=np.float32)
    A = np.asarray(A)
    W = np.ascontiguousarray(np.asarray(W, dtype=np.float32))
    a_self_c = np.ascontiguousarray(np.asarray(a_self, dtype=np.float32).reshape(D, 1))
    a_neigh_c = np.ascontiguousarray(np.asarray(a_neigh, dtype=np.float32).reshape(1, D))

    in_maps = []
    for k in range(NCORES):
        rows = slice(k * R, (k + 1) * R)
        in_maps.append({
            "feat_t": np.ascontiguousarray(features[rows, :].T),
            "a_t": A[rows, :].T.astype(np.float32),
            "w": W,
            "a_self": a_self_c,
            "a_neigh": a_neigh_c,
        })

    res = run_bass_kernel_spmd(nc, in_maps, list(range(NCORES)))
    h = np.concatenate([res.results[k]["h"] for k in range(NCORES)], axis=0)
    return h.astype(np.float32)


# revision 6
# speedup vs baseline: 1.4096x; 1.4096x over previous
"""GAT layer (nn_GATLayer) on 8 Trainium2 NeuronCores via Bass/Tile.

Reference computation (N=8192, F=512, D=64):
    z = features @ W                      # [N, D]
    s = z @ a_self; t = z @ a_neigh       # [N, 1]
    e[i,j] = leakyrelu(s[i] + t[j], 0.2)
    attention = softmax(e + mask(A), axis=1)   # mask: -1e12 where A<=0
    h = attention @ z                     # [N, D]

Sharding: row-shard the N x N attention across 8 cores (1024 rows each).
Each core computes z for its own feature rows, all-gathers z augmented
with a ones column (so the softmax denominator falls out of the PV
matmul) and a t column, then streams its [8192 x 1024] transposed block
of A while computing E[j, i] and accumulating
    H_aug[d, i] = sum_j z_aug[j, d] * E[j, i]
on the PE. Row 64 of H_aug is the softmax denominator; the epilogue
transposes H_aug back, multiplies by its reciprocal, and stores h.

Key tricks:
  * Scores are kept transposed ([j partitions, i free]) so both the
    softmax reduction and the PV contraction run over j on the PE
    partition axis - no on-chip transposes of the big matrix.
  * exp(leakyrelu(s_i + t_j)) = max(exp(s_i)exp(t_j),
    exp(.2 s_i)exp(.2 t_j)) by monotonicity, and exp separates, so the
    per-tile work is one scaled copy (ACT), one fused mult+max (DVE
    scalar_tensor_tensor), and one mask multiply (DVE/GpSimd split) -
    no per-tile transcendentals beyond the prologue exponentials.
  * Masking multiplies by A in {0,1} after exp instead of adding -1e12
    before it (exactly zeroes masked entries); A arrives as float32.
  * PV matmuls run in float32r (full-rate fp32 on the PE).
  * z_aug rows are padded to 80 floats so collective/DMA rows stay
    64-byte aligned.
"""

import sys

sys.path.insert(0, "/opt/trn_rl_repo")

import numpy as np

N, F, D = 8192, 512, 64
NCORES = 8
R = N // NCORES          # rows per core (1024)
JC = N // 128            # j-chunks (64)
DP = D + 1               # z | ones  (65)
TCOL = D + 1             # t column index in padded z_aug (65)
ZW = 80                  # padded z_aug width (80 floats = 320B rows)
ALPHA = 0.2

# mask-multiply engine split: chunks with (jc % 8) < GPS_SPLIT go to GpSimd
GPS_SPLIT = 5

_CACHE = {}


def _build_program():
    import concourse.bacc as bacc
    import concourse.tile as tile
    from concourse import mybir
    from concourse.masks import make_identity

    f32 = mybir.dt.float32
    f32r = mybir.dt.float32r
    Alu = mybir.AluOpType
    Act = mybir.ActivationFunctionType

    nc = bacc.Bacc("TRN2", target_bir_lowering=False, debug=False, num_devices=NCORES)

    feat_t = nc.dram_tensor("feat_t", [F, R], f32, kind="ExternalInput")
    a_t = nc.dram_tensor("a_t", [N, R], f32, kind="ExternalInput")
    w_in = nc.dram_tensor("w", [F, D], f32, kind="ExternalInput")
    a_self = nc.dram_tensor("a_self", [D, 1], f32, kind="ExternalInput")
    a_neigh = nc.dram_tensor("a_neigh", [1, D], f32, kind="ExternalInput")
    h_out = nc.dram_tensor("h", [R, D], f32, kind="ExternalOutput")

    with tile.TileContext(nc) as tc:
        with (
            tc.tile_pool(name="const", bufs=1) as cst,
            tc.tile_pool(name="dram", bufs=1, space="DRAM") as dram,
            tc.tile_pool(name="ps_main", bufs=1, space="PSUM") as ps_main,
        ):
            # ---- prologue inputs ----
            ft = cst.tile([128, 4 * R], f32)        # features^T, F on partitions
            for c in range(4):
                nc.sync.dma_start(out=ft[:, c * R:(c + 1) * R],
                                  in_=feat_t[c * 128:(c + 1) * 128, :])
            w_sb = cst.tile([128, 4 * D], f32)
            for c in range(4):
                nc.sync.dma_start(out=w_sb[:, c * D:(c + 1) * D],
                                  in_=w_in[c * 128:(c + 1) * 128, :])
            asf = cst.tile([D, 1], f32)
            nc.sync.dma_start(out=asf[:], in_=a_self[:])
            anr = cst.tile([1, D], f32)
            nc.sync.dma_start(out=anr[:], in_=a_neigh[:])
            ones1 = cst.tile([1, 128], f32)
            nc.vector.memset(ones1[:], 1.0)

            hp = ps_main.tile([DP, R], f32)          # H_aug accumulator

            with tc.tile_pool(name="ps_pro", bufs=2, space="PSUM") as ps_pro:
                # a_neigh broadcast across partitions: [128, D]
                pan = ps_pro.tile([128, D], f32, tag="pro")
                nc.tensor.matmul(pan[:], ones1[:], anr[:], start=True, stop=True)
                anb = cst.tile([128, D], f32)
                nc.vector.tensor_copy(anb[:], pan[:])

                # ---- z_aug_local = [z | 1 | t | 0pad]  ([R, ZW]) ----
                zaug_local = dram.tile([R, ZW], f32)
                for ib in range(R // 128):
                    psz = ps_pro.tile([128, D], f32, tag="pro")
                    for c in range(4):
                        nc.tensor.matmul(
                            psz[:],
                            ft[:, c * R + ib * 128: c * R + (ib + 1) * 128],
                            w_sb[:, c * D:(c + 1) * D],
                            start=(c == 0), stop=(c == 3),
                        )
                    zb = cst.tile([128, ZW], f32, tag="zb")
                    nc.vector.memset(zb[:], 0.0)
                    nc.vector.tensor_copy(zb[:, 0:D], psz[:])
                    nc.vector.memset(zb[:, D:D + 1], 1.0)
                    # t for this block into column TCOL
                    tscr = cst.tile([128, D], f32, tag="tscr")
                    nc.vector.tensor_tensor(tscr[:], zb[:, 0:D], anb[:], Alu.mult)
                    nc.vector.tensor_reduce(
                        zb[:, TCOL:TCOL + 1], tscr[:], mybir.AxisListType.X, Alu.add)
                    nc.sync.dma_start(
                        out=zaug_local[ib * 128:(ib + 1) * 128, :], in_=zb[:])

                # ---- all-gather z_aug (one 64B-aligned collective) ----
                zaug_full = dram.tile([N, ZW], f32, addr_space="Shared")
                nc.gpsimd.collective_compute(
                    "AllGather", Alu.bypass,
                    replica_groups=[list(range(NCORES))],
                    ins=[zaug_local.opt()], outs=[zaug_full.opt()],
                )
                zf = cst.tile([128, JC, ZW], f32)    # z_aug, j-chunked
                nc.sync.dma_start(
                    out=zf[:],
                    in_=zaug_full[:].rearrange("(c p) d -> p c d", p=128))
                # exp(.8 t) and exp(.2 t) per chunk: [128, JC]
                eq = cst.tile([128, JC], f32)
                nc.scalar.activation(eq[:], zf[:, :, TCOL], Act.Exp, scale=1.0 - ALPHA)
                et2 = cst.tile([128, JC], f32)
                nc.scalar.activation(et2[:], zf[:, :, TCOL], Act.Exp, scale=ALPHA)
                # stationary z' = z_aug * exp(.2 t_j), rounded to f32r.
                # The exp(.2 s_i) column factor cancels between numerator and
                # denominator of the softmax, so it is dropped entirely.
                zf_r = cst.tile([128, JC, DP], f32r)
                for jc in range(JC):
                    nc.vector.tensor_scalar_mul(
                        zf_r[:, jc], zf[:, jc, 0:DP], et2[:, jc:jc + 1])

                # ---- z_local^T (D on partitions) -> s row -> broadcasts ----
                pzt = ps_pro.tile([D, R], f32, tag="pro")
                for c in range(4):
                    for hh in range(2):
                        nc.tensor.matmul(
                            pzt[:, hh * 512:(hh + 1) * 512],
                            w_sb[:, c * D:(c + 1) * D],
                            ft[:, c * R + hh * 512: c * R + (hh + 1) * 512],
                            start=(c == 0), stop=(c == 3),
                        )
                zt_sb = cst.tile([D, R], f32)
                nc.vector.tensor_copy(zt_sb[:], pzt[:])

                pss = ps_pro.tile([1, R], f32, tag="pro")
                for hh in range(2):
                    nc.tensor.matmul(
                        pss[:, hh * 512:(hh + 1) * 512],
                        asf[:],
                        zt_sb[:, hh * 512:(hh + 1) * 512],
                        start=True, stop=True,
                    )
                s_sb = cst.tile([1, R], f32)
                nc.vector.tensor_copy(s_sb[:], pss[:])
                psb = ps_pro.tile([128, R], f32, tag="pro")
                for hh in range(2):
                    nc.tensor.matmul(
                        psb[:, hh * 512:(hh + 1) * 512],
                        ones1[:],
                        s_sb[0:1, hh * 512:(hh + 1) * 512],
                        start=True, stop=True,
                    )
                s_bcast = cst.tile([128, R], f32)
                nc.vector.tensor_copy(s_bcast[:], psb[:])

                # exp(.8 s), broadcast across partitions
                p3 = cst.tile([128, R], f32)
                nc.scalar.activation(p3[:], s_bcast[:], Act.Exp, scale=1.0 - ALPHA)

            # ---- main loop over j-chunks ----
            # m[j,i] = max(exp(.8 s_i) exp(.8 t_j), 1);  EA = m * A
            with (
                tc.tile_pool(name="a_pool", bufs=6) as a_pool,
                tc.tile_pool(name="work", bufs=3) as work,
            ):
                for jc in range(JC):
                    at = a_pool.tile([128, R], f32, tag="at")
                    nc.sync.dma_start(
                        out=at[:], in_=a_t[jc * 128:(jc + 1) * 128, :])

                    m = work.tile([128, R], f32, tag="m")
                    nc.vector.tensor_scalar(
                        m[:], p3[:], eq[:, jc:jc + 1], 1.0,
                        Alu.mult, Alu.max)
                    ea = work.tile([128, R], f32r, tag="ea")
                    if jc % 16 < GPS_SPLIT:
                        nc.gpsimd.tensor_tensor(ea[:], m[:], at[:], Alu.mult)
                    else:
                        nc.vector.tensor_tensor(ea[:], m[:], at[:], Alu.mult)

                    for hh in range(2):
                        nc.tensor.matmul(
                            hp[:, hh * 512:(hh + 1) * 512],
                            zf_r[:, jc],
                            ea[:, hh * 512:(hh + 1) * 512],
                            start=(jc == 0), stop=(jc == JC - 1),
                        )

            # ---- epilogue: transpose H_aug, normalize, store ----
            with (
                tc.tile_pool(name="ps_epi", bufs=2, space="PSUM") as ps_epi,
                tc.tile_pool(name="epi", bufs=2) as epi,
            ):
                h_sb = cst.tile([DP, R], f32)
                nc.vector.tensor_copy(h_sb[:], hp[:])
                ident = cst.tile([DP, DP], f32)
                make_identity(nc, ident[:])
                for b in range(R // 128):
                    trp = ps_epi.tile([128, DP], f32, tag="trp")
                    nc.tensor.transpose(
                        trp[:], h_sb[:, b * 128:(b + 1) * 128], ident[:])
                    rec = epi.tile([128, 1], f32, tag="rec")
                    nc.vector.reciprocal(rec[:], trp[:, D:DP])
                    hb = epi.tile([128, D], f32, tag="hb")
                    nc.vector.tensor_scalar_mul(hb[:], trp[:, 0:D], rec[:, 0:1])
                    nc.sync.dma_start(
                        out=h_out[b * 128:(b + 1) * 128, :], in_=hb[:])

    nc.compile()
    return nc


def _get_program():
    if "nc" not in _CACHE:
        _CACHE["nc"] = _build_program()
    return _CACHE["nc"]


def kernel(features, A, W, a_self, a_neigh):
    from concourse.bass_utils import run_bass_kernel_spmd

    nc = _get_program()

    features = np.asarray(features, dtype=np.float32)
    A = np.asarray(A)
    W = np.ascontiguousarray(np.asarray(W, dtype=np.float32))
    a_self_c = np.ascontiguousarray(np.asarray(a_self, dtype=np.float32).reshape(D, 1))
    a_neigh_c = np.ascontiguousarray(np.asarray(a_neigh, dtype=np.float32).reshape(1, D))

    in_maps = []
    for k in range(NCORES):
        rows = slice(k * R, (k + 1) * R)
        in_maps.append({
            "feat_t": np.ascontiguousarray(features[rows, :].T),
            "a_t": A[rows, :].T.astype(np.float32),
            "w": W,
            "a_self": a_self_c,
            "a_neigh": a_neigh_c,
        })

    res = run_bass_kernel_spmd(nc, in_maps, list(range(NCORES)))
    h = np.concatenate([res.results[k]["h"] for k in range(NCORES)], axis=0)
    return h.astype(np.float32)


# revision 8
# speedup vs baseline: 1.9997x; 1.4187x over previous
"""GAT layer (nn_GATLayer) on 8 Trainium2 NeuronCores via Bass/Tile.

Reference computation (N=8192, F=512, D=64):
    z = features @ W                      # [N, D]
    s = z @ a_self; t = z @ a_neigh       # [N, 1]
    e[i,j] = leakyrelu(s[i] + t[j], 0.2)
    attention = softmax(e + mask(A), axis=1)   # mask: -1e12 where A<=0
    h = attention @ z                     # [N, D]

Row-sharded across 8 cores (1024 attention rows each), two launches:

Launch A (tiny): each core computes z_aug_local = [z | 1 | t | pad] for
its own 1024 feature rows, plus its s row. The host concatenates the
eight z_aug blocks (cheap, 2.6 MB) - this replaces an on-device
AllGather whose rendezvous barrier alone cost ~47 us.

Launch B (main): each core streams its [8192 x 1024] transposed block
of A while computing mask weights and accumulating
    H_aug[d, i] = sum_j z'_aug[j, d] * (m * A)[j, i]
on the PE. Row 64 of H_aug is the softmax denominator; the epilogue
transposes H_aug back, multiplies by its reciprocal, and stores h.

Key algebra: with e = s_i + t_j,
    exp(leakyrelu(e)) = exp(.2 e) * max(exp(.8 e), 1)
                      = exp(.2 s_i) * exp(.2 t_j) * m[j,i],
    m = max(exp(.8 s_i) exp(.8 t_j), 1) = exp(relu(.8 e)).
The exp(.2 t_j) factor is folded into the stationary z'_aug =
z_aug * exp(.2 t_j); the exp(.2 s_i) factor is constant per column i of
H_aug and cancels between numerator and denominator of the softmax, so
it is dropped entirely. Per-tile work is therefore just:
    m  - either ACT Relu+Exp (chained activations, exact) or one DVE
         dual-op tensor_scalar max(p3 * eq_j, 1), split by chunk to
         balance engines;
    EA - one tensor_tensor multiply by the 0/1 mask (DVE/GpSimd split).

Other tricks:
  * Scores stay transposed ([j partitions, i free]) so the softmax
    reduction and PV contraction are both over j on the PE partition
    axis - no on-chip transposes of the big matrix.
  * Masking multiplies by A in {0,1} after exp (exactly zeroes masked
    entries). A ships as bf16 (0/1 are exact) halving mask DMA; the
    whole E pipeline and PV matmul run in bf16 against an fp32 PSUM.
  * A-tile DMAs alternate between two DGE queues (sync / scalar).
  * z_aug rows are padded to 80 floats so DMA rows stay 64B-aligned.
"""

import sys

sys.path.insert(0, "/opt/trn_rl_repo")

import numpy as np

N, F, D = 8192, 512, 64
NCORES = 8
R = N // NCORES          # rows per core (1024)
JC = N // 128            # j-chunks (64)
DP = D + 1               # z | ones  (65)
TCOL = D + 1             # t column index in padded z_aug (65)
ZW = 80                  # padded z_aug width (80 floats = 320B rows)
ALPHA = 0.2

_CACHE = {}


def _build_launch_a():
    """Per-core z_aug_local = [z | 1 | t | pad] ([R, ZW]) and s row."""
    import concourse.bacc as bacc
    import concourse.tile as tile
    from concourse import mybir

    f32 = mybir.dt.float32
    Alu = mybir.AluOpType

    nc = bacc.Bacc("TRN2", target_bir_lowering=False, debug=False, num_devices=NCORES)

    feat_t = nc.dram_tensor("feat_t", [F, R], f32, kind="ExternalInput")
    w_in = nc.dram_tensor("w", [F, D], f32, kind="ExternalInput")
    a_self = nc.dram_tensor("a_self", [D, 1], f32, kind="ExternalInput")
    a_neigh = nc.dram_tensor("a_neigh", [1, D], f32, kind="ExternalInput")
    za_out = nc.dram_tensor("za", [R, ZW], f32, kind="ExternalOutput")
    s_out = nc.dram_tensor("s", [1, R], f32, kind="ExternalOutput")

    with tile.TileContext(nc) as tc:
        with (
            tc.tile_pool(name="sb", bufs=1) as cst,
            tc.tile_pool(name="ps", bufs=2, space="PSUM") as ps,
        ):
            ft = cst.tile([128, 4 * R], f32)
            for c in range(4):
                nc.sync.dma_start(out=ft[:, c * R:(c + 1) * R],
                                  in_=feat_t[c * 128:(c + 1) * 128, :])
            w_sb = cst.tile([128, 4 * D], f32)
            for c in range(4):
                nc.sync.dma_start(out=w_sb[:, c * D:(c + 1) * D],
                                  in_=w_in[c * 128:(c + 1) * 128, :])
            asf = cst.tile([D, 1], f32)
            nc.sync.dma_start(out=asf[:], in_=a_self[:])
            anr = cst.tile([1, D], f32)
            nc.sync.dma_start(out=anr[:], in_=a_neigh[:])
            ones1 = cst.tile([1, 128], f32)
            nc.vector.memset(ones1[:], 1.0)

            pan = ps.tile([128, D], f32, tag="pro")
            nc.tensor.matmul(pan[:], ones1[:], anr[:], start=True, stop=True)
            anb = cst.tile([128, D], f32)
            nc.vector.tensor_copy(anb[:], pan[:])

            for ib in range(R // 128):
                psz = ps.tile([128, D], f32, tag="pro")
                for c in range(4):
                    nc.tensor.matmul(
                        psz[:],
                        ft[:, c * R + ib * 128: c * R + (ib + 1) * 128],
                        w_sb[:, c * D:(c + 1) * D],
                        start=(c == 0), stop=(c == 3),
                    )
                zb = cst.tile([128, ZW], f32, tag="zb")
                nc.vector.memset(zb[:], 0.0)
                nc.vector.tensor_copy(zb[:, 0:D], psz[:])
                nc.vector.memset(zb[:, D:D + 1], 1.0)
                tscr = cst.tile([128, D], f32, tag="tscr")
                nc.vector.tensor_tensor(tscr[:], zb[:, 0:D], anb[:], Alu.mult)
                nc.vector.tensor_reduce(
                    zb[:, TCOL:TCOL + 1], tscr[:], mybir.AxisListType.X, Alu.add)
                nc.sync.dma_start(
                    out=za_out[ib * 128:(ib + 1) * 128, :], in_=zb[:])

            # z_local^T (D on partitions) -> s row
            pzt = ps.tile([D, R], f32, tag="pro")
            for c in range(4):
                for hh in range(2):
                    nc.tensor.matmul(
                        pzt[:, hh * 512:(hh + 1) * 512],
                        w_sb[:, c * D:(c + 1) * D],
                        ft[:, c * R + hh * 512: c * R + (hh + 1) * 512],
                        start=(c == 0), stop=(c == 3),
                    )
            zt_sb = cst.tile([D, R], f32)
            nc.vector.tensor_copy(zt_sb[:], pzt[:])
            pss = ps.tile([1, R], f32, tag="pro")
            for hh in range(2):
                nc.tensor.matmul(
                    pss[:, hh * 512:(hh + 1) * 512],
                    asf[:],
                    zt_sb[:, hh * 512:(hh + 1) * 512],
                    start=True, stop=True,
                )
            s_sb = cst.tile([1, R], f32)
            nc.vector.tensor_copy(s_sb[:], pss[:])
            nc.sync.dma_start(out=s_out[:], in_=s_sb[:])

    nc.compile()
    return nc


# m-engine split: chunks with (jc % 16) < M_ACT_SPLIT compute m on ACT
M_ACT_SPLIT = 5
# mask-multiply split: chunks with (jc % 16) >= 16 - GPS_SPLIT go to GpSimd
GPS_SPLIT = 3


def _build_launch_b():
    import concourse.bacc as bacc
    import concourse.tile as tile
    from concourse import mybir
    from concourse.masks import make_identity

    f32 = mybir.dt.float32
    bf16 = mybir.dt.bfloat16
    Alu = mybir.AluOpType
    Act = mybir.ActivationFunctionType

    nc = bacc.Bacc("TRN2", target_bir_lowering=False, debug=False, num_devices=NCORES)

    a_t = nc.dram_tensor("a_t", [N, R], bf16, kind="ExternalInput")
    zaf = nc.dram_tensor("zaf", [N, ZW], f32, kind="ExternalInput")
    s_in = nc.dram_tensor("s", [1, R], f32, kind="ExternalInput")
    h_out = nc.dram_tensor("h", [R, D], f32, kind="ExternalOutput")

    with tile.TileContext(nc) as tc:
        with (
            tc.tile_pool(name="const", bufs=1) as cst,
            tc.tile_pool(name="ps_main", bufs=1, space="PSUM") as ps_main,
        ):
            hp = ps_main.tile([DP, R], f32)          # H_aug accumulator

            zf = cst.tile([128, JC, ZW], f32)        # z_aug, j-chunked
            nc.sync.dma_start(
                out=zf[:], in_=zaf[:].rearrange("(c p) d -> p c d", p=128))
            s_row = cst.tile([1, R], f32)
            nc.sync.dma_start(out=s_row[:], in_=s_in[:])
            ones1 = cst.tile([1, 128], f32)
            nc.vector.memset(ones1[:], 1.0)

            # t8 = .8 t, eq = exp(.8 t), et2 = exp(.2 t): [128, JC]
            t8 = cst.tile([128, JC], f32)
            nc.scalar.activation(t8[:], zf[:, :, TCOL], Act.Identity,
                                 scale=1.0 - ALPHA)
            eq = cst.tile([128, JC], f32)
            nc.scalar.activation(eq[:], zf[:, :, TCOL], Act.Exp,
                                 scale=1.0 - ALPHA)
            et2 = cst.tile([128, JC], f32)
            nc.scalar.activation(et2[:], zf[:, :, TCOL], Act.Exp, scale=ALPHA)

            # stationary z' = z_aug * exp(.2 t_j), rounded to bf16.
            # The exp(.2 s_i) column factor cancels in the softmax.
            zf_b = cst.tile([128, JC, DP], bf16)
            for jc in range(JC):
                nc.vector.tensor_scalar_mul(
                    zf_b[:, jc], zf[:, jc, 0:DP], et2[:, jc:jc + 1])

            # s broadcast across partitions; p3 = exp(.8 s) in bf16
            with tc.tile_pool(name="ps_pro", bufs=2, space="PSUM") as ps_pro:
                psb = ps_pro.tile([128, R], f32, tag="pro")
                for hh in range(2):
                    nc.tensor.matmul(
                        psb[:, hh * 512:(hh + 1) * 512],
                        ones1[:],
                        s_row[0:1, hh * 512:(hh + 1) * 512],
                        start=True, stop=True,
                    )
                s_bcast = cst.tile([128, R], f32)
                nc.vector.tensor_copy(s_bcast[:], psb[:])
            p3 = cst.tile([128, R], bf16)
            nc.scalar.activation(p3[:], s_bcast[:], Act.Exp, scale=1.0 - ALPHA)

            # ---- main loop over j-chunks ----
            # m[j,i] = exp(relu(.8(s_i + t_j))) = max(exp(.8 s)exp(.8 t), 1)
            with (
                tc.tile_pool(name="a_pool", bufs=8) as a_pool,
                tc.tile_pool(name="work", bufs=3) as work,
            ):
                for jc in range(JC):
                    at = a_pool.tile([128, R], bf16, tag="at")
                    dma_eng = nc.sync if jc % 2 == 0 else nc.scalar
                    dma_eng.dma_start(
                        out=at[:], in_=a_t[jc * 128:(jc + 1) * 128, :])

                    m = work.tile([128, R], bf16, tag="m")
                    if jc % 16 < M_ACT_SPLIT:
                        u = work.tile([128, R], f32, tag="u")
                        nc.scalar.activation(
                            u[:], s_bcast[:], Act.Relu,
                            bias=t8[:, jc:jc + 1], scale=1.0 - ALPHA)
                        nc.scalar.activation(m[:], u[:], Act.Exp)
                    else:
                        nc.vector.tensor_scalar(
                            m[:], p3[:], eq[:, jc:jc + 1], 1.0,
                            Alu.mult, Alu.max)
                    ea = work.tile([128, R], bf16, tag="ea")
                    if jc % 16 >= 16 - GPS_SPLIT:
                        nc.gpsimd.tensor_tensor(ea[:], m[:], at[:], Alu.mult)
                    else:
                        nc.vector.tensor_tensor(ea[:], m[:], at[:], Alu.mult)

                    for hh in range(2):
                        nc.tensor.matmul(
                            hp[:, hh * 512:(hh + 1) * 512],
                            zf_b[:, jc],
                            ea[:, hh * 512:(hh + 1) * 512],
                            start=(jc == 0), stop=(jc == JC - 1),
                        )

            # ---- epilogue: transpose H_aug, normalize, store ----
            with (
                tc.tile_pool(name="ps_epi", bufs=2, space="PSUM") as ps_epi,
                tc.tile_pool(name="epi", bufs=2) as epi,
            ):
                h_sb = cst.tile([DP, R], f32)
                nc.vector.tensor_copy(h_sb[:], hp[:])
                ident = cst.tile([DP, DP], f32)
                make_identity(nc, ident[:])
                for b in range(R // 128):
                    trp = ps_epi.tile([128, DP], f32, tag="trp")
                    nc.tensor.transpose(
                        trp[:], h_sb[:, b * 128:(b + 1) * 128], ident[:])
                    rec = epi.tile([128, 1], f32, tag="rec")
                    nc.vector.reciprocal(rec[:], trp[:, D:DP])
                    hb = epi.tile([128, D], f32, tag="hb")
                    nc.vector.tensor_scalar_mul(hb[:], trp[:, 0:D], rec[:, 0:1])
                    nc.sync.dma_start(
                        out=h_out[b * 128:(b + 1) * 128, :], in_=hb[:])

    nc.compile()
    return nc


def _get_programs():
    if "a" not in _CACHE:
        _CACHE["a"] = _build_launch_a()
        _CACHE["b"] = _build_launch_b()
    return _CACHE["a"], _CACHE["b"]


def _mask_to_bf16(block):
    """0/1 int mask -> bf16 exactly, fast (bit pattern 0x3F80 = 1.0)."""
    import ml_dtypes
    bits = (block != 0).astype(np.uint16) * np.uint16(0x3F80)
    return bits.view(ml_dtypes.bfloat16)


def kernel(features, A, W, a_self, a_neigh):
    from concourse.bass_utils import run_bass_kernel_spmd

    nca, ncb = _get_programs()

    features = np.asarray(features, dtype=np.float32)
    A = np.asarray(A)
    W = np.ascontiguousarray(np.asarray(W, dtype=np.float32))
    a_self_c = np.ascontiguousarray(np.asarray(a_self, dtype=np.float32).reshape(D, 1))
    a_neigh_c = np.ascontiguousarray(np.asarray(a_neigh, dtype=np.float32).reshape(1, D))

    in_a = []
    for k in range(NCORES):
        rows = slice(k * R, (k + 1) * R)
        in_a.append({
            "feat_t": np.ascontiguousarray(features[rows, :].T),
            "w": W,
            "a_self": a_self_c,
            "a_neigh": a_neigh_c,
        })
    res_a = run_bass_kernel_spmd(nca, in_a, list(range(NCORES))).results
    zaf = np.concatenate([res_a[k]["za"] for k in range(NCORES)], axis=0)

    in_b = []
    for k in range(NCORES):
        rows = slice(k * R, (k + 1) * R)
        in_b.append({
            "a_t": _mask_to_bf16(np.asarray(A[rows, :]).T),
            "zaf": zaf,
            "s": res_a[k]["s"],
        })
    res_b = run_bass_kernel_spmd(ncb, in_b, list(range(NCORES))).results
    h = np.concatenate([res_b[k]["h"] for k in range(NCORES)], axis=0)
    return h.astype(np.float32)


# revision 11
# speedup vs baseline: 2.2641x; 1.1322x over previous
"""GAT layer (nn_GATLayer) on 8 Trainium2 NeuronCores via Bass/Tile.

Reference computation (N=8192, F=512, D=64):
    z = features @ W                      # [N, D]
    s = z @ a_self; t = z @ a_neigh       # [N, 1]
    e[i,j] = leakyrelu(s[i] + t[j], 0.2)
    attention = softmax(e + mask(A), axis=1)   # mask: -1e12 where A<=0
    h = attention @ z                     # [N, D]

Row-sharded across 8 cores (1024 attention rows each), two launches:

Launch A (tiny): each core computes z_aug_local = [z | 1 | t | pad] for
its own 1024 feature rows, plus its s row. The host concatenates the
eight z_aug blocks (cheap, 2.6 MB) - this replaces an on-device
AllGather whose rendezvous barrier alone cost ~47 us.

Launch B (main): each core streams its [8192 x 1024] transposed block
of A while computing mask weights and accumulating
    H_aug[d, i] = sum_j z'_aug[j, d] * (m * A)[j, i]
on the PE. Row 64 of H_aug is the softmax denominator; the epilogue
transposes H_aug back, multiplies by its reciprocal, and stores h.

Key algebra: with e = s_i + t_j,
    exp(leakyrelu(e)) = exp(.2 e) * max(exp(.8 e), 1)
                      = exp(.2 s_i) * exp(.2 t_j) * m[j,i],
    m = max(exp(.8 s_i) exp(.8 t_j), 1) = exp(relu(.8 e)).
The exp(.2 t_j) factor is folded into the stationary z'_aug =
z_aug * exp(.2 t_j); the exp(.2 s_i) factor is constant per column i of
H_aug and cancels between numerator and denominator of the softmax, so
it is dropped entirely. Per-tile work is therefore just:
    m  - either ACT Relu+Exp (chained activations, exact) or one DVE
         dual-op tensor_scalar max(p3 * eq_j, 1), split by chunk to
         balance engines;
    EA - one tensor_tensor multiply by the 0/1 mask (DVE/GpSimd split).

Other tricks:
  * Scores stay transposed ([j partitions, i free]) so the softmax
    reduction and PV contraction are both over j on the PE partition
    axis - no on-chip transposes of the big matrix.
  * Masking multiplies by A in {0,1} after exp (exactly zeroes masked
    entries). A ships as float16 (0/1 are exact) halving mask DMA; the
    whole E pipeline and PV matmul run in float16 against an fp32 PSUM
    (f16 keeps 10 mantissa bits at the same PE/DVE rates as bf16).
  * A-tile DMAs rotate across three DGE queues (sync/scalar/gpsimd);
    even/odd chunks accumulate into two separate PSUM banks so the PE
    is not serialized on a single accumulation chain.
  * z_aug rows are padded to 80 floats so DMA rows stay 64B-aligned.
"""

import sys

sys.path.insert(0, "/opt/trn_rl_repo")

import numpy as np

N, F, D = 8192, 512, 64
NCORES = 8
R = N // NCORES          # rows per core (1024)
JC = N // 128            # j-chunks (64)
DP = D + 1               # z | ones  (65)
TCOL = D + 1             # t column index in padded z_aug (65)
ZW = 80                  # padded z_aug width (80 floats = 320B rows)
ALPHA = 0.2

_CACHE = {}


def _build_launch_a():
    """Per-core z_aug_local = [z | 1 | t | pad] ([R, ZW]) and s row."""
    import concourse.bacc as bacc
    import concourse.tile as tile
    from concourse import mybir

    f32 = mybir.dt.float32
    f32r = mybir.dt.float32r
    Alu = mybir.AluOpType

    nc = bacc.Bacc("TRN2", target_bir_lowering=False, debug=False, num_devices=NCORES)

    feat_t = nc.dram_tensor("feat_t", [F, R], f32, kind="ExternalInput")
    w_in = nc.dram_tensor("w", [F, D], f32, kind="ExternalInput")
    a_self = nc.dram_tensor("a_self", [D, 1], f32, kind="ExternalInput")
    a_neigh = nc.dram_tensor("a_neigh", [1, D], f32, kind="ExternalInput")
    za_out = nc.dram_tensor("za", [R, ZW], f32, kind="ExternalOutput")
    s_out = nc.dram_tensor("s", [1, R], f32, kind="ExternalOutput")

    with tile.TileContext(nc) as tc:
        with (
            tc.tile_pool(name="sb", bufs=1) as cst,
            tc.tile_pool(name="ps", bufs=2, space="PSUM") as ps,
        ):
            ft = cst.tile([128, 4 * R], f32)
            for c in range(4):
                nc.sync.dma_start(out=ft[:, c * R:(c + 1) * R],
                                  in_=feat_t[c * 128:(c + 1) * 128, :])
            w_sb = cst.tile([128, 4 * D], f32)
            for c in range(4):
                nc.sync.dma_start(out=w_sb[:, c * D:(c + 1) * D],
                                  in_=w_in[c * 128:(c + 1) * 128, :])
            asf = cst.tile([D, 1], f32)
            nc.sync.dma_start(out=asf[:], in_=a_self[:])
            anr = cst.tile([1, D], f32)
            nc.sync.dma_start(out=anr[:], in_=a_neigh[:])
            ones1 = cst.tile([1, 128], f32)
            nc.vector.memset(ones1[:], 1.0)

            pan = ps.tile([128, D], f32, tag="pro")
            nc.tensor.matmul(pan[:], ones1[:], anr[:], start=True, stop=True)
            anb = cst.tile([128, D], f32)
            nc.vector.tensor_copy(anb[:], pan[:])

            zb = cst.tile([128, R // 128, ZW], f32)
            nc.vector.memset(zb[:], 0.0)
            for ib in range(R // 128):
                psz = ps.tile([128, D], f32, tag="pro")
                for c in range(4):
                    nc.tensor.matmul(
                        psz[:],
                        ft[:, c * R + ib * 128: c * R + (ib + 1) * 128],
                        w_sb[:, c * D:(c + 1) * D],
                        start=(c == 0), stop=(c == 3),
                    )
                nc.vector.tensor_copy(zb[:, ib, 0:D], psz[:])
            nc.vector.memset(zb[:, :, D:D + 1], 1.0)
            tscr = cst.tile([128, R // 128, D], f32)
            for ib in range(R // 128):
                nc.vector.tensor_tensor(
                    tscr[:, ib], zb[:, ib, 0:D], anb[:], Alu.mult)
            nc.vector.tensor_reduce(
                zb[:, :, TCOL:TCOL + 1], tscr[:], mybir.AxisListType.X, Alu.add)
            nc.sync.dma_start(
                out=za_out[:].rearrange("(c p) d -> p c d", p=128), in_=zb[:])

            # z_local^T (D on partitions) -> s row (f32r: s only feeds
            # exp(.8 s) so reduced-precision matmuls are plenty)
            ftr = cst.tile([128, 4 * R], f32r)
            nc.vector.tensor_copy(ftr[:], ft[:])
            wr = cst.tile([128, 4 * D], f32r)
            nc.vector.tensor_copy(wr[:], w_sb[:])
            pzt = ps.tile([D, R], f32, tag="pro")
            for c in range(4):
                for hh in range(2):
                    nc.tensor.matmul(
                        pzt[:, hh * 512:(hh + 1) * 512],
                        wr[:, c * D:(c + 1) * D],
                        ftr[:, c * R + hh * 512: c * R + (hh + 1) * 512],
                        start=(c == 0), stop=(c == 3),
                    )
            zt_sb = cst.tile([D, R], f32)
            nc.vector.tensor_copy(zt_sb[:], pzt[:])
            pss = ps.tile([1, R], f32, tag="pro")
            for hh in range(2):
                nc.tensor.matmul(
                    pss[:, hh * 512:(hh + 1) * 512],
                    asf[:],
                    zt_sb[:, hh * 512:(hh + 1) * 512],
                    start=True, stop=True,
                )
            s_sb = cst.tile([1, R], f32)
            nc.vector.tensor_copy(s_sb[:], pss[:])
            nc.sync.dma_start(out=s_out[:], in_=s_sb[:])

    nc.compile()
    return nc


# m-engine split: chunks with (jc % 16) < M_ACT_SPLIT compute m on ACT
M_ACT_SPLIT = 5
# mask-multiply split: chunks with (jc % 16) >= 16 - GPS_SPLIT go to GpSimd
GPS_SPLIT = 3


def _build_launch_b():
    import concourse.bacc as bacc
    import concourse.tile as tile
    from concourse import mybir
    from concourse.masks import make_identity

    f32 = mybir.dt.float32
    f16 = mybir.dt.float16
    Alu = mybir.AluOpType
    Act = mybir.ActivationFunctionType

    nc = bacc.Bacc("TRN2", target_bir_lowering=False, debug=False, num_devices=NCORES)

    a_t = nc.dram_tensor("a_t", [N, R], f16, kind="ExternalInput")
    zaf = nc.dram_tensor("zaf", [N, ZW], f32, kind="ExternalInput")
    s_in = nc.dram_tensor("s", [1, R], f32, kind="ExternalInput")
    h_out = nc.dram_tensor("h", [R, D], f32, kind="ExternalOutput")

    with tile.TileContext(nc) as tc:
        with (
            tc.tile_pool(name="const", bufs=1) as cst,
            tc.tile_pool(name="ps_main", bufs=1, space="PSUM") as ps_main,
        ):
            hp0 = ps_main.tile([DP, R], f32)         # H_aug accumulator (even)
            hp1 = ps_main.tile([DP, R], f32)         # H_aug accumulator (odd)

            zf = cst.tile([128, JC, ZW], f32)        # z_aug, j-chunked
            nc.sync.dma_start(
                out=zf[:], in_=zaf[:].rearrange("(c p) d -> p c d", p=128))
            s_row = cst.tile([1, R], f32)
            nc.sync.dma_start(out=s_row[:], in_=s_in[:])
            ones1 = cst.tile([1, 128], f32)
            nc.vector.memset(ones1[:], 1.0)

            # t8 = .8 t, eq = exp(.8 t), et2 = exp(.2 t): [128, JC]
            t8 = cst.tile([128, JC], f32)
            nc.scalar.activation(t8[:], zf[:, :, TCOL], Act.Identity,
                                 scale=1.0 - ALPHA)
            eq = cst.tile([128, JC], f32)
            nc.scalar.activation(eq[:], zf[:, :, TCOL], Act.Exp,
                                 scale=1.0 - ALPHA)
            et2 = cst.tile([128, JC], f32)
            nc.scalar.activation(et2[:], zf[:, :, TCOL], Act.Exp, scale=ALPHA)

            # stationary z' = z_aug * exp(.2 t_j), rounded to bf16.
            # The exp(.2 s_i) column factor cancels in the softmax.
            zf_b = cst.tile([128, JC, DP], f16)
            for jc in range(JC):
                nc.vector.tensor_scalar_mul(
                    zf_b[:, jc], zf[:, jc, 0:DP], et2[:, jc:jc + 1])

            # s broadcast across partitions; p3 = exp(.8 s) in bf16
            with tc.tile_pool(name="ps_pro", bufs=2, space="PSUM") as ps_pro:
                psb = ps_pro.tile([128, R], f32, tag="pro")
                for hh in range(2):
                    nc.tensor.matmul(
                        psb[:, hh * 512:(hh + 1) * 512],
                        ones1[:],
                        s_row[0:1, hh * 512:(hh + 1) * 512],
                        start=True, stop=True,
                    )
                s_bcast = cst.tile([128, R], f32)
                nc.vector.tensor_copy(s_bcast[:], psb[:])
            p3 = cst.tile([128, R], f16)
            nc.scalar.activation(p3[:], s_bcast[:], Act.Exp, scale=1.0 - ALPHA)

            # ---- main loop over j-chunks ----
            # m[j,i] = exp(relu(.8(s_i + t_j))) = max(exp(.8 s)exp(.8 t), 1)
            with (
                tc.tile_pool(name="a_pool", bufs=8) as a_pool,
                tc.tile_pool(name="work", bufs=3) as work,
            ):
                dma_engines = [nc.sync, nc.scalar, nc.gpsimd]
                for jc in range(JC):
                    at = a_pool.tile([128, R], f16, tag="at")
                    dma_engines[jc % 3].dma_start(
                        out=at[:], in_=a_t[jc * 128:(jc + 1) * 128, :])

                    m = work.tile([128, R], f16, tag="m")
                    if jc % 3 == 1:
                        u = work.tile([128, R], f32, tag="u")
                        nc.scalar.activation(
                            u[:], s_bcast[:], Act.Relu,
                            bias=t8[:, jc:jc + 1], scale=1.0 - ALPHA)
                        nc.scalar.activation(m[:], u[:], Act.Exp)
                    else:
                        nc.vector.tensor_scalar(
                            m[:], p3[:], eq[:, jc:jc + 1], 1.0,
                            Alu.mult, Alu.max)
                    ea = work.tile([128, R], f16, tag="ea")
                    if jc % 4 == 2:
                        nc.gpsimd.tensor_tensor(ea[:], m[:], at[:], Alu.mult)
                    else:
                        nc.vector.tensor_tensor(ea[:], m[:], at[:], Alu.mult)

                    hp = hp0 if jc % 2 == 0 else hp1
                    for hh in range(2):
                        nc.tensor.matmul(
                            hp[:, hh * 512:(hh + 1) * 512],
                            zf_b[:, jc],
                            ea[:, hh * 512:(hh + 1) * 512],
                            start=(jc < 2), stop=(jc >= JC - 2),
                        )

            # ---- epilogue: transpose H_aug, normalize, store ----
            with (
                tc.tile_pool(name="ps_epi", bufs=2, space="PSUM") as ps_epi,
                tc.tile_pool(name="epi", bufs=2) as epi,
            ):
                h_sb = cst.tile([DP, R], f32)
                nc.vector.tensor_copy(h_sb[:], hp0[:])
                nc.vector.tensor_tensor(h_sb[:], h_sb[:], hp1[:], Alu.add)
                ident = cst.tile([DP, DP], f32)
                make_identity(nc, ident[:])
                for b in range(R // 128):
                    trp = ps_epi.tile([128, DP], f32, tag="trp")
                    nc.tensor.transpose(
                        trp[:], h_sb[:, b * 128:(b + 1) * 128], ident[:])
                    rec = epi.tile([128, 1], f32, tag="rec")
                    nc.vector.reciprocal(rec[:], trp[:, D:DP])
                    hb = epi.tile([128, D], f32, tag="hb")
                    nc.vector.tensor_scalar_mul(hb[:], trp[:, 0:D], rec[:, 0:1])
                    nc.sync.dma_start(
                        out=h_out[b * 128:(b + 1) * 128, :], in_=hb[:])

    nc.compile()
    return nc


def _get_programs():
    if "a" not in _CACHE:
        _CACHE["a"] = _build_launch_a()
        _CACHE["b"] = _build_launch_b()
    return _CACHE["a"], _CACHE["b"]


def _mask_to_f16(block):
    """0/1 int mask -> float16 exactly, fast (bit pattern 0x3C00 = 1.0)."""
    bits = (block != 0).astype(np.uint16) * np.uint16(0x3C00)
    return bits.view(np.float16)


def kernel(features, A, W, a_self, a_neigh):
    from concourse.bass_utils import run_bass_kernel_spmd

    nca, ncb = _get_programs()

    features = np.asarray(features, dtype=np.float32)
    A = np.asarray(A)
    W = np.ascontiguousarray(np.asarray(W, dtype=np.float32))
    a_self_c = np.ascontiguousarray(np.asarray(a_self, dtype=np.float32).reshape(D, 1))
    a_neigh_c = np.ascontiguousarray(np.asarray(a_neigh, dtype=np.float32).reshape(1, D))

    in_a = []
    for k in range(NCORES):
        rows = slice(k * R, (k + 1) * R)
        in_a.append({
            "feat_t": np.ascontiguousarray(features[rows, :].T),
            "w": W,
            "a_self": a_self_c,
            "a_neigh": a_neigh_c,
        })
    res_a = run_bass_kernel_spmd(nca, in_a, list(range(NCORES))).results
    zaf = np.concatenate([res_a[k]["za"] for k in range(NCORES)], axis=0)

    in_b = []
    for k in range(NCORES):
        rows = slice(k * R, (k + 1) * R)
        in_b.append({
            "a_t": _mask_to_f16(np.asarray(A[rows, :]).T),
            "zaf": zaf,
            "s": res_a[k]["s"],
        })
    res_b = run_bass_kernel_spmd(ncb, in_b, list(range(NCORES))).results
    h = np.concatenate([res_b[k]["h"] for k in range(NCORES)], axis=0)
    return h.astype(np.float32)


# revision 12
# speedup vs baseline: 2.2674x; 1.0014x over previous
"""GAT layer (nn_GATLayer) on 8 Trainium2 NeuronCores via Bass/Tile.

Reference computation (N=8192, F=512, D=64):
    z = features @ W                      # [N, D]
    s = z @ a_self; t = z @ a_neigh       # [N, 1]
    e[i,j] = leakyrelu(s[i] + t[j], 0.2)
    attention = softmax(e + mask(A), axis=1)   # mask: -1e12 where A<=0
    h = attention @ z                     # [N, D]

Row-sharded across 8 cores (1024 attention rows each), two launches:

Launch A (tiny): each core computes z_aug_local = [z | 1 | t | pad] for
its own 1024 feature rows, plus its s row. The host concatenates the
eight z_aug blocks (cheap, 2.6 MB) - this replaces an on-device
AllGather whose rendezvous barrier alone cost ~47 us.

Launch B (main): each core streams its [8192 x 1024] transposed block
of A while computing mask weights and accumulating
    H_aug[d, i] = sum_j z'_aug[j, d] * (m * A)[j, i]
on the PE. Row 64 of H_aug is the softmax denominator; the epilogue
transposes H_aug back, multiplies by its reciprocal, and stores h.

Key algebra: with e = s_i + t_j,
    exp(leakyrelu(e)) = exp(.2 e) * max(exp(.8 e), 1)
                      = exp(.2 s_i) * exp(.2 t_j) * m[j,i],
    m = max(exp(.8 s_i) exp(.8 t_j), 1) = exp(relu(.8 e)).
The exp(.2 t_j) factor is folded into the stationary z'_aug =
z_aug * exp(.2 t_j); the exp(.2 s_i) factor is constant per column i of
H_aug and cancels between numerator and denominator of the softmax, so
it is dropped entirely. Per-tile work is therefore just:
    m  - either ACT Relu+Exp (chained activations, exact) or one DVE
         dual-op tensor_scalar max(p3 * eq_j, 1), split by chunk to
         balance engines;
    EA - one tensor_tensor multiply by the 0/1 mask (DVE/GpSimd split).

Other tricks:
  * Scores stay transposed ([j partitions, i free]) so the softmax
    reduction and PV contraction are both over j on the PE partition
    axis - no on-chip transposes of the big matrix.
  * Masking multiplies by A in {0,1} after exp (exactly zeroes masked
    entries). A ships as float16 (0/1 are exact) halving mask DMA; the
    whole E pipeline and PV matmul run in float16 against an fp32 PSUM
    (f16 keeps 10 mantissa bits at the same PE/DVE rates as bf16).
  * A-tile DMAs rotate across three DGE queues (sync/scalar/gpsimd);
    even/odd chunks accumulate into two separate PSUM banks so the PE
    is not serialized on a single accumulation chain.
  * z_aug rows are padded to 80 floats so DMA rows stay 64B-aligned.
"""

import sys

sys.path.insert(0, "/opt/trn_rl_repo")

import numpy as np

N, F, D = 8192, 512, 64
NCORES = 8
R = N // NCORES          # rows per core (1024)
JC = N // 128            # j-chunks (64)
DP = D + 1               # z | ones  (65)
TCOL = D + 1             # t column index in padded z_aug (65)
ZW = 80                  # padded z_aug width (80 floats = 320B rows)
ALPHA = 0.2

_CACHE = {}


def _build_launch_a():
    """Per-core z_aug_local = [z | 1 | t | pad] ([R, ZW]) and s row."""
    import concourse.bacc as bacc
    import concourse.tile as tile
    from concourse import mybir

    f32 = mybir.dt.float32
    f32r = mybir.dt.float32r
    Alu = mybir.AluOpType

    nc = bacc.Bacc("TRN2", target_bir_lowering=False, debug=False, num_devices=NCORES)

    feat_t = nc.dram_tensor("feat_t", [F, R], f32, kind="ExternalInput")
    w_in = nc.dram_tensor("w", [F, D], f32, kind="ExternalInput")
    a_self = nc.dram_tensor("a_self", [D, 1], f32, kind="ExternalInput")
    a_neigh = nc.dram_tensor("a_neigh", [1, D], f32, kind="ExternalInput")
    za_out = nc.dram_tensor("za", [R, ZW], f32, kind="ExternalOutput")
    s_out = nc.dram_tensor("s", [1, R], f32, kind="ExternalOutput")
    t_out = nc.dram_tensor("t", [R, 1], f32, kind="ExternalOutput")

    with tile.TileContext(nc) as tc:
        with (
            tc.tile_pool(name="sb", bufs=1) as cst,
            tc.tile_pool(name="ps", bufs=2, space="PSUM") as ps,
        ):
            ft = cst.tile([128, 4 * R], f32)
            for c in range(4):
                nc.sync.dma_start(out=ft[:, c * R:(c + 1) * R],
                                  in_=feat_t[c * 128:(c + 1) * 128, :])
            w_sb = cst.tile([128, 4 * D], f32)
            for c in range(4):
                nc.sync.dma_start(out=w_sb[:, c * D:(c + 1) * D],
                                  in_=w_in[c * 128:(c + 1) * 128, :])
            asf = cst.tile([D, 1], f32)
            nc.sync.dma_start(out=asf[:], in_=a_self[:])
            anr = cst.tile([1, D], f32)
            nc.sync.dma_start(out=anr[:], in_=a_neigh[:])
            ones1 = cst.tile([1, 128], f32)
            nc.vector.memset(ones1[:], 1.0)

            pan = ps.tile([128, D], f32, tag="pro")
            nc.tensor.matmul(pan[:], ones1[:], anr[:], start=True, stop=True)
            anb = cst.tile([128, D], f32)
            nc.vector.tensor_copy(anb[:], pan[:])

            zb = cst.tile([128, R // 128, ZW], f32)
            nc.vector.memset(zb[:], 0.0)
            for ib in range(R // 128):
                psz = ps.tile([128, D], f32, tag="pro")
                for c in range(4):
                    nc.tensor.matmul(
                        psz[:],
                        ft[:, c * R + ib * 128: c * R + (ib + 1) * 128],
                        w_sb[:, c * D:(c + 1) * D],
                        start=(c == 0), stop=(c == 3),
                    )
                nc.vector.tensor_copy(zb[:, ib, 0:D], psz[:])
            nc.vector.memset(zb[:, :, D:D + 1], 1.0)
            tscr = cst.tile([128, R // 128, D], f32)
            for ib in range(R // 128):
                nc.vector.tensor_tensor(
                    tscr[:, ib], zb[:, ib, 0:D], anb[:], Alu.mult)
            nc.vector.tensor_reduce(
                zb[:, :, TCOL:TCOL + 1], tscr[:], mybir.AxisListType.X, Alu.add)
            nc.sync.dma_start(
                out=za_out[:].rearrange("(c p) d -> p c d", p=128), in_=zb[:])
            nc.sync.dma_start(
                out=t_out[:].rearrange("(c p) one -> p c one", p=128),
                in_=zb[:, :, TCOL:TCOL + 1])

            # z_local^T (D on partitions) -> s row (f32r: s only feeds
            # exp(.8 s) so reduced-precision matmuls are plenty)
            ftr = cst.tile([128, 4 * R], f32r)
            nc.vector.tensor_copy(ftr[:], ft[:])
            wr = cst.tile([128, 4 * D], f32r)
            nc.vector.tensor_copy(wr[:], w_sb[:])
            pzt = ps.tile([D, R], f32, tag="pro")
            for c in range(4):
                for hh in range(2):
                    nc.tensor.matmul(
                        pzt[:, hh * 512:(hh + 1) * 512],
                        wr[:, c * D:(c + 1) * D],
                        ftr[:, c * R + hh * 512: c * R + (hh + 1) * 512],
                        start=(c == 0), stop=(c == 3),
                    )
            zt_sb = cst.tile([D, R], f32)
            nc.vector.tensor_copy(zt_sb[:], pzt[:])
            pss = ps.tile([1, R], f32, tag="pro")
            for hh in range(2):
                nc.tensor.matmul(
                    pss[:, hh * 512:(hh + 1) * 512],
                    asf[:],
                    zt_sb[:, hh * 512:(hh + 1) * 512],
                    start=True, stop=True,
                )
            s_sb = cst.tile([1, R], f32)
            nc.vector.tensor_copy(s_sb[:], pss[:])
            nc.sync.dma_start(out=s_out[:], in_=s_sb[:])

    nc.compile()
    return nc


# m-engine split: chunks with (jc % 16) < M_ACT_SPLIT compute m on ACT
M_ACT_SPLIT = 5
# mask-multiply split: chunks with (jc % 16) >= 16 - GPS_SPLIT go to GpSimd
GPS_SPLIT = 3


def _build_launch_b():
    import concourse.bacc as bacc
    import concourse.tile as tile
    from concourse import mybir
    from concourse.masks import make_identity

    f32 = mybir.dt.float32
    f16 = mybir.dt.float16
    Alu = mybir.AluOpType
    Act = mybir.ActivationFunctionType

    nc = bacc.Bacc("TRN2", target_bir_lowering=False, debug=False, num_devices=NCORES)

    a_t = nc.dram_tensor("a_t", [N, R], f16, kind="ExternalInput")
    zaf = nc.dram_tensor("zaf", [N, ZW], f32, kind="ExternalInput")
    s_in = nc.dram_tensor("s", [1, R], f32, kind="ExternalInput")
    t_in = nc.dram_tensor("t", [N, 1], f32, kind="ExternalInput")
    h_out = nc.dram_tensor("h", [R, D], f32, kind="ExternalOutput")

    with tile.TileContext(nc) as tc:
        with (
            tc.tile_pool(name="const", bufs=1) as cst,
            tc.tile_pool(name="ps_main", bufs=1, space="PSUM") as ps_main,
        ):
            hp0 = ps_main.tile([DP, R], f32)         # H_aug accumulator (even)
            hp1 = ps_main.tile([DP, R], f32)         # H_aug accumulator (odd)

            # small inputs first: t (32KB) and s unblock the score chain
            # long before the 2.6MB z_aug DMA completes
            tt = cst.tile([128, JC, 1], f32)
            nc.scalar.dma_start(
                out=tt[:], in_=t_in[:].rearrange("(c p) one -> p c one", p=128))
            s_row = cst.tile([1, R], f32)
            nc.scalar.dma_start(out=s_row[:], in_=s_in[:])
            zf = cst.tile([128, JC, ZW], f32)        # z_aug, j-chunked
            nc.sync.dma_start(
                out=zf[:], in_=zaf[:].rearrange("(c p) d -> p c d", p=128))
            ones1 = cst.tile([1, 128], f32)
            nc.vector.memset(ones1[:], 1.0)

            # t8 = .8 t, eq = exp(.8 t), et2 = exp(.2 t): [128, JC]
            t8 = cst.tile([128, JC], f32)
            nc.scalar.activation(t8[:], tt[:, :, 0], Act.Identity,
                                 scale=1.0 - ALPHA)
            eq = cst.tile([128, JC], f32)
            nc.scalar.activation(eq[:], tt[:, :, 0], Act.Exp,
                                 scale=1.0 - ALPHA)
            et2 = cst.tile([128, JC], f32)
            nc.scalar.activation(et2[:], tt[:, :, 0], Act.Exp, scale=ALPHA)

            # stationary z' = z_aug * exp(.2 t_j), rounded to bf16.
            # The exp(.2 s_i) column factor cancels in the softmax.
            zf_b = cst.tile([128, JC, DP], f16)
            for jc in range(JC):
                nc.vector.tensor_scalar_mul(
                    zf_b[:, jc], zf[:, jc, 0:DP], et2[:, jc:jc + 1])

            # s broadcast across partitions; p3 = exp(.8 s) in bf16
            with tc.tile_pool(name="ps_pro", bufs=2, space="PSUM") as ps_pro:
                psb = ps_pro.tile([128, R], f32, tag="pro")
                for hh in range(2):
                    nc.tensor.matmul(
                        psb[:, hh * 512:(hh + 1) * 512],
                        ones1[:],
                        s_row[0:1, hh * 512:(hh + 1) * 512],
                        start=True, stop=True,
                    )
                s_bcast = cst.tile([128, R], f32)
                nc.vector.tensor_copy(s_bcast[:], psb[:])
            p3 = cst.tile([128, R], f16)
            nc.scalar.activation(p3[:], s_bcast[:], Act.Exp, scale=1.0 - ALPHA)

            # ---- main loop over j-chunks ----
            # m[j,i] = exp(relu(.8(s_i + t_j))) = max(exp(.8 s)exp(.8 t), 1)
            with (
                tc.tile_pool(name="a_pool", bufs=8) as a_pool,
                tc.tile_pool(name="work", bufs=3) as work,
            ):
                dma_engines = [nc.sync, nc.scalar, nc.gpsimd]
                for jc in range(JC):
                    at = a_pool.tile([128, R], f16, tag="at")
                    dma_engines[jc % 3].dma_start(
                        out=at[:], in_=a_t[jc * 128:(jc + 1) * 128, :])

                    m = work.tile([128, R], f16, tag="m")
                    if jc % 3 == 1:
                        u = work.tile([128, R], f32, tag="u")
                        nc.scalar.activation(
                            u[:], s_bcast[:], Act.Relu,
                            bias=t8[:, jc:jc + 1], scale=1.0 - ALPHA)
                        nc.scalar.activation(m[:], u[:], Act.Exp)
                    else:
                        nc.vector.tensor_scalar(
                            m[:], p3[:], eq[:, jc:jc + 1], 1.0,
                            Alu.mult, Alu.max)
                    ea = work.tile([128, R], f16, tag="ea")
                    if jc % 8 == 2:
                        nc.gpsimd.tensor_tensor(ea[:], m[:], at[:], Alu.mult)
                    else:
                        nc.vector.tensor_tensor(ea[:], m[:], at[:], Alu.mult)

                    hp = hp0 if jc % 2 == 0 else hp1
                    for hh in range(2):
                        nc.tensor.matmul(
                            hp[:, hh * 512:(hh + 1) * 512],
                            zf_b[:, jc],
                            ea[:, hh * 512:(hh + 1) * 512],
                            start=(jc < 2), stop=(jc >= JC - 2),
                        )

            # ---- epilogue: transpose H_aug, normalize, store ----
            with (
                tc.tile_pool(name="ps_epi", bufs=2, space="PSUM") as ps_epi,
                tc.tile_pool(name="epi", bufs=2) as epi,
            ):
                h_sb = cst.tile([DP, R], f32)
                nc.vector.tensor_copy(h_sb[:], hp0[:])
                nc.vector.tensor_tensor(h_sb[:], h_sb[:], hp1[:], Alu.add)
                ident = cst.tile([DP, DP], f32)
                make_identity(nc, ident[:])
                for b in range(R // 128):
                    trp = ps_epi.tile([128, DP], f32, tag="trp")
                    nc.tensor.transpose(
                        trp[:], h_sb[:, b * 128:(b + 1) * 128], ident[:])
                    rec = epi.tile([128, 1], f32, tag="rec")
                    nc.vector.reciprocal(rec[:], trp[:, D:DP])
                    hb = epi.tile([128, D], f32, tag="hb")
                    nc.vector.tensor_scalar_mul(hb[:], trp[:, 0:D], rec[:, 0:1])
                    nc.sync.dma_start(
                        out=h_out[b * 128:(b + 1) * 128, :], in_=hb[:])

    nc.compile()
    return nc


def _get_programs():
    if "a" not in _CACHE:
        _CACHE["a"] = _build_launch_a()
        _CACHE["b"] = _build_launch_b()
    return _CACHE["a"], _CACHE["b"]


def _mask_to_f16(block):
    """0/1 int mask -> float16 exactly, fast (bit pattern 0x3C00 = 1.0)."""
    bits = (block != 0).astype(np.uint16) * np.uint16(0x3C00)
    return bits.view(np.float16)


def kernel(features, A, W, a_self, a_neigh):
    from concourse.bass_utils import run_bass_kernel_spmd

    nca, ncb = _get_programs()

    features = np.asarray(features, dtype=np.float32)
    A = np.asarray(A)
    W = np.ascontiguousarray(np.asarray(W, dtype=np.float32))
    a_self_c = np.ascontiguousarray(np.asarray(a_self, dtype=np.float32).reshape(D, 1))
    a_neigh_c = np.ascontiguousarray(np.asarray(a_neigh, dtype=np.float32).reshape(1, D))

    in_a = []
    for k in range(NCORES):
        rows = slice(k * R, (k + 1) * R)
        in_a.append({
            "feat_t": np.ascontiguousarray(features[rows, :].T),
            "w": W,
            "a_self": a_self_c,
            "a_neigh": a_neigh_c,
        })
    res_a = run_bass_kernel_spmd(nca, in_a, list(range(NCORES))).results
    zaf = np.concatenate([res_a[k]["za"] for k in range(NCORES)], axis=0)
    t_full = np.concatenate([res_a[k]["t"] for k in range(NCORES)], axis=0)

    in_b = []
    for k in range(NCORES):
        rows = slice(k * R, (k + 1) * R)
        in_b.append({
            "a_t": _mask_to_f16(np.asarray(A[rows, :]).T),
            "zaf": zaf,
            "s": res_a[k]["s"],
            "t": t_full,
        })
    res_b = run_bass_kernel_spmd(ncb, in_b, list(range(NCORES))).results
    h = np.concatenate([res_b[k]["h"] for k in range(NCORES)], axis=0)
    return h.astype(np.float32)
